# revision 8
# baseline (speedup 1.0000x reference)
"""MoE gate (LLaDA2) routing kernel for 8 Trainium2 NeuronCores.

Strategy: token-parallel over 8 cores (2048 tokens/core). Router GEMM split
into a fp16 main pass (xh16 @ w16) plus a single fp8e4m3 DoubleRow GEMM that
computes BOTH cross-correction terms (xh@wl + xl@wh) via augmented vectors
u=[x|xl*2^12], v=[wl*2^19|w*2^7] over a 8192-long contraction at 0.5
cycles/row — recovering ~15.5-bit effective operand precision (needed for
exact top-k ordering) at ~60% of the PE time of an fp16 3-term split.
Grouped top-k routing on-chip: DVE max8/max_index/match_replace with cheap
elementwise stages offloaded to GPSIMD.
"""
import sys
for p in ("/opt/trn_rl_repo", "/root/.axon_site/_ro/trn_rl_repo"):
    if p not in sys.path:
        sys.path.append(p)

import numpy as np

T, H, E = 16384, 4096, 256
NCORES = 8
TPC = T // NCORES          # tokens per core: 2048
NTILES = TPC // 128        # 16 row tiles
KCH = H // 128             # 32 contraction chunks (fp16 main pass)
SCH = 2 * KCH              # 64 fp8 subchunks (augmented contraction)
G = 8                      # expert groups
GS = E // G                # 32 experts/group
K = 8                      # top-k
BIG = 2.0 ** 100
NEG = -1.0e4
CSCALE = 2.0 ** -19        # undo fp8 segment scaling at combine time

_cache = {}


def _build():
    import concourse.bacc as bacc
    import concourse.bass as bass
    import concourse.mybir as mybir
    from concourse import tile

    dt = mybir.dt
    Alu = mybir.AluOpType
    Act = mybir.ActivationFunctionType
    Ax = mybir.AxisListType
    DR = mybir.MatmulPerfMode.DoubleRow

    nc = bacc.Bacc("TRN2", target_bir_lowering=False, debug=False,
                   num_devices=NCORES)

    xh_d = nc.dram_tensor("xh", [NTILES, 128, KCH, 128], dt.float16, kind="ExternalInput")
    u8_d = nc.dram_tensor("u8", [NTILES, 128, SCH, 128], dt.float8e4, kind="ExternalInput")
    w16_d = nc.dram_tensor("w16", [128, KCH, E], dt.float16, kind="ExternalInput")
    v8_d = nc.dram_tensor("v8", [128, SCH, E], dt.float8e4, kind="ExternalInput")
    btab_d = nc.dram_tensor("btab", [128, E], dt.float32, kind="ExternalInput")
    w_out = nc.dram_tensor("w_out", [TPC, K], dt.float32, kind="ExternalOutput")
    i_out = nc.dram_tensor("i_out", [TPC, K], dt.uint32, kind="ExternalOutput")

    def bc_mid(ap8, n=8):
        # [128, m] -> [128, n(bcast), m]
        return bass.AP(ap8.tensor, ap8.offset, [list(ap8.ap[0]), [0, n], list(ap8.ap[1])])

    with tile.TileContext(nc) as tc:
        with (
            tc.tile_pool(name="wpool", bufs=1) as wpool,
            tc.tile_pool(name="xpool", bufs=4) as xpool,
            tc.tile_pool(name="ppool", bufs=4, space="PSUM") as ppool,
            tc.tile_pool(name="spool", bufs=2) as spool,
            tc.tile_pool(name="tpool", bufs=2) as tpool,
            tc.tile_pool(name="opool", bufs=1) as opool,
        ):
            # DMA issue order: w16 half 0, xh/u8 tile 0, v8, w16 half 1 —
            # so tile-0 main matmuls and the fp8 pass start early.
            KS = KCH // 2
            w16a = wpool.tile([128, KS * E], dt.float16, tag="w16a")
            w16b = wpool.tile([128, KS * E], dt.float16, tag="w16b")
            w16_flat = w16_d[:].rearrange("p k e -> p (k e)")
            for s in range(2):
                sz = KS * E // 2
                nc.sync.dma_start(w16a[:, s * sz:(s + 1) * sz],
                                  w16_flat[:, s * sz:(s + 1) * sz])
            x0 = xpool.tile([128, KCH * 128], dt.float16, tag="x")
            nc.sync.dma_start(x0[:], xh_d[0].rearrange("p k t -> p (k t)"))
            u0 = xpool.tile([128, SCH * 128], dt.float8e4, tag="u")
            nc.sync.dma_start(u0[:], u8_d[0].rearrange("p s t -> p (s t)"))
            v8 = wpool.tile([128, SCH * E], dt.float8e4, tag="v8")
            v8_flat = v8_d[:].rearrange("p s e -> p (s e)")
            for s in range(4):
                sz = SCH * E // 4
                nc.sync.dma_start(v8[:, s * sz:(s + 1) * sz],
                                  v8_flat[:, s * sz:(s + 1) * sz])
            for s in range(2):
                sz = KS * E // 2
                nc.sync.dma_start(w16b[:, s * sz:(s + 1) * sz],
                                  w16_flat[:, KS * E + s * sz:KS * E + (s + 1) * sz])
            btab = wpool.tile([128, E], dt.float32, tag="btab")
            nc.sync.dma_start(btab[:], btab_d[:])

            out_w = opool.tile([128, NTILES * K], dt.float32, tag="ow")
            out_i = opool.tile([128, NTILES * K], dt.uint32, tag="oi")

            for i in range(NTILES):
                if i == 0:
                    x, u = x0, u0
                else:
                    x = xpool.tile([128, KCH * 128], dt.float16, tag="x")
                    nc.sync.dma_start(x[:], xh_d[i].rearrange("p k t -> p (k t)"))
                    u = xpool.tile([128, SCH * 128], dt.float8e4, tag="u")
                    nc.sync.dma_start(u[:], u8_d[i].rearrange("p s t -> p (s t)"))

                psm = ppool.tile([128, E], dt.float32, tag="psm")
                for k in range(KCH):
                    wt = w16a if k < KS else w16b
                    kk = k % KS
                    nc.tensor.matmul(psm[:],
                                     lhsT=x[:, k * 128:(k + 1) * 128],
                                     rhs=wt[:, kk * E:(kk + 1) * E],
                                     start=(k == 0), stop=(k == KCH - 1))

                psc = ppool.tile([128, E], dt.float32, tag="psc")
                u3 = u[:].rearrange("p (s t) -> p s t", s=SCH)
                v3 = v8[:].rearrange("p (s e) -> p s e", s=SCH)
                for j in range(SCH // 2):
                    nc.tensor.matmul(psc[:],
                                     lhsT=u3[:, 2 * j:2 * j + 2, :],
                                     rhs=v3[:, 2 * j:2 * j + 2, :],
                                     start=(j == 0), stop=(j == SCH // 2 - 1),
                                     perf_mode=DR)

                # --- combine + routing epilogue (ACT + GPSIMD + DVE) ---
                # GPSIMD cannot read PSUM: evacuate both psums via ACT, add on GPSIMD
                s1 = spool.tile([128, E], dt.float32, tag="s1")
                nc.scalar.copy(s1[:], psm[:])
                s2 = spool.tile([128, E], dt.float32, tag="s2")
                nc.scalar.mul(s2[:], psc[:], CSCALE)
                comb = spool.tile([128, E], dt.float32, tag="comb")
                nc.gpsimd.tensor_tensor(comb[:], s1[:], s2[:], Alu.add)
                scores = spool.tile([128, E], dt.float32, tag="scores")
                nc.scalar.activation(scores[:], comb[:], Act.Sigmoid)

                sr = spool.tile([128, E], dt.float32, tag="sr")
                nc.gpsimd.tensor_tensor(sr[:], scores[:], btab[:], Alu.add)
                sr3 = sr[:].rearrange("p (g e) -> p g e", g=G)

                top1 = tpool.tile([128, G], dt.float32, tag="top1")
                nc.vector.tensor_reduce(top1[:], sr3, axis=Ax.X, op=Alu.max)
                mr2 = spool.tile([128, E], dt.float32, tag="mr2")
                nc.vector.match_replace(mr2[:], in_to_replace=top1[:], in_values=sr[:], imm_value=NEG)
                top2 = tpool.tile([128, G], dt.float32, tag="top2")
                nc.vector.tensor_reduce(top2[:], mr2[:].rearrange("p (g e) -> p g e", g=G), axis=Ax.X, op=Alu.max)
                gs_t = tpool.tile([128, G], dt.float32, tag="gs")
                nc.gpsimd.tensor_tensor(gs_t[:], top1[:], top2[:], Alu.add)
                g8 = tpool.tile([128, 8], dt.float32, tag="g8")
                nc.vector.max(out=g8[:], in_=gs_t[:])
                inv = tpool.tile([128, G], dt.float32, tag="inv")
                nc.vector.tensor_scalar(inv[:], gs_t[:], g8[:, 3:4], -NEG, op0=Alu.is_lt, op1=Alu.mult)
                # mask: sr -= inv (0 for kept groups, 1e4 for dropped)
                nc.gpsimd.tensor_tensor(sr3, sr3, inv[:].to_broadcast([128, G, GS]), Alu.subtract)

                vals8 = tpool.tile([128, K], dt.float32, tag="vals8")
                nc.vector.max(out=vals8[:], in_=sr[:])
                idx8 = tpool.tile([128, K], dt.uint32, tag="idx8")
                nc.vector.max_index(out=idx8[:], in_max=vals8[:], in_values=sr[:])

                # selected positions -> exactly BIG; (BIG * 2^-100) * score = score
                mr = spool.tile([128, E], dt.float32, tag="mr")
                nc.vector.match_replace(mr[:], in_to_replace=vals8[:], in_values=sr[:], imm_value=BIG)
                sel_s = spool.tile([128, E], dt.float32, tag="sel_s")
                nc.vector.scalar_tensor_tensor(sel_s[:], in0=mr[:], scalar=2.0 ** -100,
                                               in1=scores[:], op0=Alu.mult, op1=Alu.mult)

                svals8 = tpool.tile([128, K], dt.float32, tag="svals8")
                nc.vector.max(out=svals8[:], in_=sel_s[:])
                sidx8 = tpool.tile([128, K], dt.uint32, tag="sidx8")
                nc.vector.max_index(out=sidx8[:], in_max=svals8[:], in_values=sel_s[:])

                idx8f = tpool.tile([128, K], dt.float32, tag="idx8f")
                nc.vector.tensor_copy(idx8f[:], idx8[:])
                sidx8f = tpool.tile([128, K], dt.float32, tag="sidx8f")
                nc.vector.tensor_copy(sidx8f[:], sidx8[:])

                # reorder svals8 (score order) into idx8 (routing order): K x K match
                eq = tpool.tile([128, K * K], dt.float32, tag="eq")
                eq3 = eq[:].rearrange("p (k j) -> p k j", k=K)
                nc.vector.tensor_tensor(eq3, idx8f[:].to_broadcast([128, K, K]), bc_mid(sidx8f[:]), Alu.is_equal)
                prod = tpool.tile([128, K * K], dt.float32, tag="prod")
                prod3 = prod[:].rearrange("p (k j) -> p k j", k=K)
                nc.vector.tensor_tensor(prod3, eq3, bc_mid(svals8[:]), Alu.mult)
                w8 = tpool.tile([128, K], dt.float32, tag="w8")
                nc.vector.tensor_reduce(w8[:], prod3, axis=Ax.X, op=Alu.add)

                sum8 = tpool.tile([128, 1], dt.float32, tag="sum8")
                nc.vector.tensor_reduce(sum8[:], w8[:], axis=Ax.X, op=Alu.add)
                rec = tpool.tile([128, 1], dt.float32, tag="rec")
                nc.vector.reciprocal(rec[:], sum8[:])

                nc.vector.tensor_scalar(out_w[:, i * K:(i + 1) * K], w8[:], rec[:, 0:1], 2.5,
                                        op0=Alu.mult, op1=Alu.mult)
                nc.gpsimd.tensor_copy(out_i[:, i * K:(i + 1) * K], idx8[:])

            nc.sync.dma_start(w_out[:].rearrange("(i p) k -> p i k", p=128),
                              out_w[:].rearrange("p (i k) -> p i k", i=NTILES))
            nc.sync.dma_start(i_out[:].rearrange("(i p) k -> p i k", p=128),
                              out_i[:].rearrange("p (i k) -> p i k", i=NTILES))

    nc.compile()
    return nc


def _prep(hidden_states, weight, expert_bias):
    import ml_dtypes
    f8 = ml_dtypes.float8_e4m3
    x = np.ascontiguousarray(hidden_states, dtype=np.float32)
    w = np.ascontiguousarray(weight, dtype=np.float32)

    w16 = w.astype(np.float16)
    wl = w - w16.astype(np.float32)
    wl8 = (wl * 2.0 ** 19).astype(f8)       # scaled residual of w
    wh8 = (w * 2.0 ** 7).astype(f8)         # scaled w
    # [256, 4096] -> [128p, k, 256e] layouts
    w16_l = np.ascontiguousarray(w16.reshape(E, KCH, 128).transpose(2, 1, 0))
    v8 = np.concatenate([wl8.reshape(E, KCH, 128), wh8.reshape(E, KCH, 128)], axis=1)
    v8_l = np.ascontiguousarray(v8.transpose(2, 1, 0))   # [128, 64, 256]
    btab = np.ascontiguousarray(np.broadcast_to(expert_bias.astype(np.float32), (128, E)))

    in_maps = []
    for c in range(NCORES):
        xs = x[c * TPC:(c + 1) * TPC]
        xh16 = xs.astype(np.float16)
        xl = xs - xh16.astype(np.float32)
        xh8 = xs.astype(f8)
        xl8 = (xl * 2.0 ** 12).astype(f8)
        # [2048, 4096] -> [16i, 128p(h), k, 128t]
        xh_l = np.ascontiguousarray(xh16.reshape(NTILES, 128, KCH, 128).transpose(0, 3, 2, 1))
        u8 = np.concatenate([xh8.reshape(NTILES, 128, KCH, 128),
                             xl8.reshape(NTILES, 128, KCH, 128)], axis=2)
        u8_l = np.ascontiguousarray(u8.transpose(0, 3, 2, 1))  # [16, 128, 64, 128]
        in_maps.append({"xh": xh_l, "u8": u8_l, "w16": w16_l, "v8": v8_l, "btab": btab})
    return in_maps


def kernel(hidden_states, weight, expert_bias, _trace=False):
    from concourse.bass_utils import run_bass_kernel_spmd

    if "nc" not in _cache:
        _cache["nc"] = _build()
    nc = _cache["nc"]
    in_maps = _prep(hidden_states, weight, expert_bias)
    res = run_bass_kernel_spmd(nc, in_maps, core_ids=list(range(NCORES)), trace=_trace)
    _cache["last_results"] = res
    w = np.concatenate([res.results[c]["w_out"] for c in range(NCORES)], axis=0)
    idx = np.concatenate([res.results[c]["i_out"] for c in range(NCORES)], axis=0)
    return w.astype(np.float32), idx.astype(np.int32)


# revision 14
# speedup vs baseline: 1.0992x; 1.0992x over previous
"""MoE gate (LLaDA2) routing kernel for 8 Trainium2 NeuronCores.

Strategy: token-parallel over 8 cores (2048 tokens/core). Router GEMM split
into a fp16 main pass (xh16 @ w16) plus a single fp8e4m3 DoubleRow GEMM that
computes BOTH cross-correction terms (xh@wl + xl@wh) via augmented vectors
u=[x|xl*2^12], v=[wl*2^19|w*2^7] over a 8192-long contraction at 0.5
cycles/row — recovering ~15.5-bit effective operand precision (needed for
exact top-k ordering) at ~60% of the PE time of an fp16 3-term split.
Grouped top-k routing on-chip: DVE max8/max_index/match_replace with cheap
elementwise stages offloaded to GPSIMD.
"""
import sys
for p in ("/opt/trn_rl_repo", "/root/.axon_site/_ro/trn_rl_repo"):
    if p not in sys.path:
        sys.path.append(p)

import numpy as np

T, H, E = 16384, 4096, 256
NCORES = 8
TPC = T // NCORES          # tokens per core: 2048
NTILES = TPC // 128        # 16 row tiles
KCH = H // 128             # 32 contraction chunks (fp16 main pass)
SCH = 2 * KCH              # 64 fp8 subchunks (augmented contraction)
G = 8                      # expert groups
GS = E // G                # 32 experts/group
K = 8                      # top-k
BIG = 2.0 ** 100
NEG = -1.0e4
CSCALE = 2.0 ** -19        # undo fp8 segment scaling at combine time

_cache = {}


def _build():
    import concourse.bacc as bacc
    import concourse.bass as bass
    import concourse.mybir as mybir
    from concourse import tile

    dt = mybir.dt
    Alu = mybir.AluOpType
    Act = mybir.ActivationFunctionType
    Ax = mybir.AxisListType
    DR = mybir.MatmulPerfMode.DoubleRow

    nc = bacc.Bacc("TRN2", target_bir_lowering=False, debug=False,
                   num_devices=NCORES)

    xh_d = nc.dram_tensor("xh", [NTILES, 128, KCH, 128], dt.float16, kind="ExternalInput")
    u8_d = nc.dram_tensor("u8", [NTILES, 128, SCH, 128], dt.float8e4, kind="ExternalInput")
    w16_d = nc.dram_tensor("w16", [128, KCH, E], dt.float16, kind="ExternalInput")
    v8_d = nc.dram_tensor("v8", [128, SCH, E], dt.float8e4, kind="ExternalInput")
    btab_d = nc.dram_tensor("btab", [128, E], dt.float32, kind="ExternalInput")
    w_out = nc.dram_tensor("w_out", [TPC, K], dt.float32, kind="ExternalOutput")
    i_out = nc.dram_tensor("i_out", [TPC, K], dt.uint32, kind="ExternalOutput")

    def bc_mid(ap8, n=8):
        # [128, m] -> [128, n(bcast), m]
        return bass.AP(ap8.tensor, ap8.offset, [list(ap8.ap[0]), [0, n], list(ap8.ap[1])])

    with tile.TileContext(nc) as tc:
        with (
            tc.tile_pool(name="wpool", bufs=1) as wpool,
            tc.tile_pool(name="xpool", bufs=4) as xpool,
            tc.tile_pool(name="ppool", bufs=4, space="PSUM") as ppool,
            tc.tile_pool(name="spool", bufs=3) as spool,
            tc.tile_pool(name="tpool", bufs=3) as tpool,
            tc.tile_pool(name="opool", bufs=1) as opool,
        ):
            # DMA issue order: w16 half 0, xh/u8 tile 0, v8, w16 half 1 —
            # so tile-0 main matmuls and the fp8 pass start early.
            KS = KCH // 2
            w16a = wpool.tile([128, KS * E], dt.float16, tag="w16a")
            w16b = wpool.tile([128, KS * E], dt.float16, tag="w16b")
            w16_flat = w16_d[:].rearrange("p k e -> p (k e)")
            for s in range(2):
                sz = KS * E // 2
                nc.sync.dma_start(w16a[:, s * sz:(s + 1) * sz],
                                  w16_flat[:, s * sz:(s + 1) * sz])
            x0 = xpool.tile([128, KCH * 128], dt.float16, tag="x")
            nc.sync.dma_start(x0[:], xh_d[0].rearrange("p k t -> p (k t)"))
            u0 = xpool.tile([128, SCH * 128], dt.float8e4, tag="u")
            nc.sync.dma_start(u0[:], u8_d[0].rearrange("p s t -> p (s t)"))
            v8 = wpool.tile([128, SCH * E], dt.float8e4, tag="v8")
            v8_flat = v8_d[:].rearrange("p s e -> p (s e)")
            for s in range(4):
                sz = SCH * E // 4
                nc.sync.dma_start(v8[:, s * sz:(s + 1) * sz],
                                  v8_flat[:, s * sz:(s + 1) * sz])
            for s in range(2):
                sz = KS * E // 2
                nc.sync.dma_start(w16b[:, s * sz:(s + 1) * sz],
                                  w16_flat[:, KS * E + s * sz:KS * E + (s + 1) * sz])
            btab = wpool.tile([128, E], dt.float32, tag="btab")
            nc.sync.dma_start(btab[:], btab_d[:])

            out_w = opool.tile([128, NTILES * K], dt.float32, tag="ow")
            out_i = opool.tile([128, NTILES * K], dt.uint32, tag="oi")

            for i in range(NTILES):
                if i == 0:
                    x, u = x0, u0
                else:
                    x = xpool.tile([128, KCH * 128], dt.float16, tag="x")
                    nc.sync.dma_start(x[:], xh_d[i].rearrange("p k t -> p (k t)"))
                    u = xpool.tile([128, SCH * 128], dt.float8e4, tag="u")
                    nc.sync.dma_start(u[:], u8_d[i].rearrange("p s t -> p (s t)"))

                psm = ppool.tile([128, E], dt.float32, tag="psm")
                for k in range(KCH):
                    wt = w16a if k < KS else w16b
                    kk = k % KS
                    nc.tensor.matmul(psm[:],
                                     lhsT=x[:, k * 128:(k + 1) * 128],
                                     rhs=wt[:, kk * E:(kk + 1) * E],
                                     start=(k == 0), stop=(k == KCH - 1))

                psc = ppool.tile([128, E], dt.float32, tag="psc")
                u3 = u[:].rearrange("p (s t) -> p s t", s=SCH)
                v3 = v8[:].rearrange("p (s e) -> p s e", s=SCH)
                for j in range(SCH // 2):
                    nc.tensor.matmul(psc[:],
                                     lhsT=u3[:, 2 * j:2 * j + 2, :],
                                     rhs=v3[:, 2 * j:2 * j + 2, :],
                                     start=(j == 0), stop=(j == SCH // 2 - 1),
                                     perf_mode=DR)

                # --- combine + routing epilogue (ACT + GPSIMD + DVE) ---
                # DVE may read only one PSUM operand: ACT rescales psc to SBUF first
                s2 = spool.tile([128, E], dt.float32, tag="s2")
                nc.scalar.mul(s2[:], psc[:], CSCALE)
                comb = spool.tile([128, E], dt.float32, tag="comb")
                nc.vector.tensor_tensor(comb[:], s2[:], psm[:], Alu.add)
                scores = spool.tile([128, E], dt.float32, tag="scores")
                nc.scalar.activation(scores[:], comb[:], Act.Sigmoid)

                sr = spool.tile([128, E], dt.float32, tag="sr")
                nc.gpsimd.tensor_tensor(sr[:], scores[:], btab[:], Alu.add)
                sr3 = sr[:].rearrange("p (g e) -> p g e", g=G)

                top1 = tpool.tile([128, G], dt.float32, tag="top1")
                nc.vector.tensor_reduce(top1[:], sr3, axis=Ax.X, op=Alu.max)
                mr2 = spool.tile([128, E], dt.float32, tag="mr2")
                nc.vector.match_replace(mr2[:], in_to_replace=top1[:], in_values=sr[:], imm_value=NEG)
                top2 = tpool.tile([128, G], dt.float32, tag="top2")
                nc.vector.tensor_reduce(top2[:], mr2[:].rearrange("p (g e) -> p g e", g=G), axis=Ax.X, op=Alu.max)
                gs_t = tpool.tile([128, G], dt.float32, tag="gs")
                nc.vector.tensor_tensor(gs_t[:], top1[:], top2[:], Alu.add)
                g8 = tpool.tile([128, 8], dt.float32, tag="g8")
                nc.vector.max(out=g8[:], in_=gs_t[:])
                inv = tpool.tile([128, G], dt.float32, tag="inv")
                nc.vector.tensor_scalar(inv[:], gs_t[:], g8[:, 3:4], -NEG, op0=Alu.is_lt, op1=Alu.mult)
                # mask: sr -= inv (0 for kept groups, 1e4 for dropped)
                nc.gpsimd.tensor_tensor(sr3, sr3, inv[:].to_broadcast([128, G, GS]), Alu.subtract)

                vals8 = tpool.tile([128, K], dt.float32, tag="vals8")
                nc.vector.max(out=vals8[:], in_=sr[:])
                idx8 = tpool.tile([128, K], dt.uint32, tag="idx8")
                nc.vector.max_index(out=idx8[:], in_max=vals8[:], in_values=sr[:])

                # selected positions -> exactly BIG; (BIG * 2^-100) * score = score
                mr = spool.tile([128, E], dt.float32, tag="mr")
                nc.vector.match_replace(mr[:], in_to_replace=vals8[:], in_values=sr[:], imm_value=BIG)
                sel_s = spool.tile([128, E], dt.float32, tag="sel_s")
                nc.vector.scalar_tensor_tensor(sel_s[:], in0=mr[:], scalar=2.0 ** -100,
                                               in1=scores[:], op0=Alu.mult, op1=Alu.mult)

                svals8 = tpool.tile([128, K], dt.float32, tag="svals8")
                nc.vector.max(out=svals8[:], in_=sel_s[:])
                sidx8 = tpool.tile([128, K], dt.uint32, tag="sidx8")
                nc.vector.max_index(out=sidx8[:], in_max=svals8[:], in_values=sel_s[:])

                idx8f = tpool.tile([128, K], dt.float32, tag="idx8f")
                nc.vector.tensor_copy(idx8f[:], idx8[:])
                sidx8f = tpool.tile([128, K], dt.float32, tag="sidx8f")
                nc.vector.tensor_copy(sidx8f[:], sidx8[:])

                # reorder svals8 (score order) into idx8 (routing order): K x K match
                eq = tpool.tile([128, K * K], dt.float32, tag="eq")
                eq3 = eq[:].rearrange("p (k j) -> p k j", k=K)
                nc.vector.tensor_tensor(eq3, idx8f[:].to_broadcast([128, K, K]), bc_mid(sidx8f[:]), Alu.is_equal)
                prod = tpool.tile([128, K * K], dt.float32, tag="prod")
                prod3 = prod[:].rearrange("p (k j) -> p k j", k=K)
                nc.vector.tensor_tensor(prod3, eq3, bc_mid(svals8[:]), Alu.mult)
                w8 = tpool.tile([128, K], dt.float32, tag="w8")
                nc.vector.tensor_reduce(w8[:], prod3, axis=Ax.X, op=Alu.add)

                sum8 = tpool.tile([128, 1], dt.float32, tag="sum8")
                nc.vector.tensor_reduce(sum8[:], w8[:], axis=Ax.X, op=Alu.add)
                rec = tpool.tile([128, 1], dt.float32, tag="rec")
                nc.vector.reciprocal(rec[:], sum8[:])

                nc.vector.tensor_scalar(out_w[:, i * K:(i + 1) * K], w8[:], rec[:, 0:1], 2.5,
                                        op0=Alu.mult, op1=Alu.mult)
                nc.gpsimd.tensor_copy(out_i[:, i * K:(i + 1) * K], idx8[:])

                # stream results out every 4 tiles so the final DMA is tiny
                if i % 4 == 3:
                    g = i // 4
                    wo4 = w_out[:].rearrange("(i p) k -> p i k", p=128)[:, 4 * g:4 * g + 4, :]
                    io4 = i_out[:].rearrange("(i p) k -> p i k", p=128)[:, 4 * g:4 * g + 4, :]
                    nc.sync.dma_start(wo4, out_w[:, 32 * g:32 * (g + 1)].rearrange("p (i k) -> p i k", i=4))
                    nc.sync.dma_start(io4, out_i[:, 32 * g:32 * (g + 1)].rearrange("p (i k) -> p i k", i=4))

    nc.compile()
    return nc


def _prep(hidden_states, weight, expert_bias):
    import ml_dtypes
    f8 = ml_dtypes.float8_e4m3
    x = np.ascontiguousarray(hidden_states, dtype=np.float32)
    w = np.ascontiguousarray(weight, dtype=np.float32)

    w16 = w.astype(np.float16)
    wl = w - w16.astype(np.float32)
    wl8 = (wl * 2.0 ** 19).astype(f8)       # scaled residual of w
    wh8 = (w * 2.0 ** 7).astype(f8)         # scaled w
    # [256, 4096] -> [128p, k, 256e] layouts
    w16_l = np.ascontiguousarray(w16.reshape(E, KCH, 128).transpose(2, 1, 0))
    v8 = np.concatenate([wl8.reshape(E, KCH, 128), wh8.reshape(E, KCH, 128)], axis=1)
    v8_l = np.ascontiguousarray(v8.transpose(2, 1, 0))   # [128, 64, 256]
    btab = np.ascontiguousarray(np.broadcast_to(expert_bias.astype(np.float32), (128, E)))

    in_maps = []
    for c in range(NCORES):
        xs = x[c * TPC:(c + 1) * TPC]
        xh16 = xs.astype(np.float16)
        xl = xs - xh16.astype(np.float32)
        xh8 = xs.astype(f8)
        xl8 = (xl * 2.0 ** 12).astype(f8)
        # [2048, 4096] -> [16i, 128p(h), k, 128t]
        xh_l = np.ascontiguousarray(xh16.reshape(NTILES, 128, KCH, 128).transpose(0, 3, 2, 1))
        u8 = np.concatenate([xh8.reshape(NTILES, 128, KCH, 128),
                             xl8.reshape(NTILES, 128, KCH, 128)], axis=2)
        u8_l = np.ascontiguousarray(u8.transpose(0, 3, 2, 1))  # [16, 128, 64, 128]
        in_maps.append({"xh": xh_l, "u8": u8_l, "w16": w16_l, "v8": v8_l, "btab": btab})
    return in_maps


def kernel(hidden_states, weight, expert_bias, _trace=False):
    from concourse.bass_utils import run_bass_kernel_spmd

    if "nc" not in _cache:
        _cache["nc"] = _build()
    nc = _cache["nc"]
    in_maps = _prep(hidden_states, weight, expert_bias)
    res = run_bass_kernel_spmd(nc, in_maps, core_ids=list(range(NCORES)), trace=_trace)
    _cache["last_results"] = res
    w = np.concatenate([res.results[c]["w_out"] for c in range(NCORES)], axis=0)
    idx = np.concatenate([res.results[c]["i_out"] for c in range(NCORES)], axis=0)
    return w.astype(np.float32), idx.astype(np.int32)


# revision 15
# speedup vs baseline: 1.1172x; 1.0163x over previous
"""MoE gate (LLaDA2) routing kernel for 8 Trainium2 NeuronCores.

Strategy: token-parallel over 8 cores (2048 tokens/core). Router GEMM split
into a fp16 main pass (xh16 @ w16) plus a single fp8e4m3 DoubleRow GEMM that
computes BOTH cross-correction terms (xh@wl + xl@wh) via augmented vectors
u=[x|xl*2^12], v=[wl*2^19|w*2^7] over a 8192-long contraction at 0.5
cycles/row — recovering ~15.5-bit effective operand precision (needed for
exact top-k ordering) at ~60% of the PE time of an fp16 3-term split.
Grouped top-k routing on-chip: DVE max8/max_index/match_replace with cheap
elementwise stages offloaded to GPSIMD.
"""
import sys
for p in ("/opt/trn_rl_repo", "/root/.axon_site/_ro/trn_rl_repo"):
    if p not in sys.path:
        sys.path.append(p)

import numpy as np

T, H, E = 16384, 4096, 256
NCORES = 8
TPC = T // NCORES          # tokens per core: 2048
NTILES = TPC // 128        # 16 row tiles
KCH = H // 128             # 32 contraction chunks (fp16 main pass)
SCH = 2 * KCH              # 64 fp8 subchunks (augmented contraction)
G = 8                      # expert groups
GS = E // G                # 32 experts/group
K = 8                      # top-k
BIG = 2.0 ** 100
NEG = -1.0e4
CSCALE = 2.0 ** -19        # undo fp8 segment scaling at combine time

_cache = {}


def _build():
    import concourse.bacc as bacc
    import concourse.bass as bass
    import concourse.mybir as mybir
    from concourse import tile

    dt = mybir.dt
    Alu = mybir.AluOpType
    Act = mybir.ActivationFunctionType
    Ax = mybir.AxisListType
    DR = mybir.MatmulPerfMode.DoubleRow

    nc = bacc.Bacc("TRN2", target_bir_lowering=False, debug=False,
                   num_devices=NCORES)

    xh_d = nc.dram_tensor("xh", [NTILES, 128, KCH, 128], dt.float16, kind="ExternalInput")
    u8_d = nc.dram_tensor("u8", [NTILES, 128, SCH, 128], dt.float8e4, kind="ExternalInput")
    w16_d = nc.dram_tensor("w16", [128, KCH, E], dt.float16, kind="ExternalInput")
    v8_d = nc.dram_tensor("v8", [128, SCH, E], dt.float8e4, kind="ExternalInput")
    btab_d = nc.dram_tensor("btab", [128, E], dt.float32, kind="ExternalInput")
    w_out = nc.dram_tensor("w_out", [TPC, K], dt.float32, kind="ExternalOutput")
    i_out = nc.dram_tensor("i_out", [TPC, K], dt.uint32, kind="ExternalOutput")

    def bc_mid(ap8, n=8):
        # [128, m] -> [128, n(bcast), m]
        return bass.AP(ap8.tensor, ap8.offset, [list(ap8.ap[0]), [0, n], list(ap8.ap[1])])

    with tile.TileContext(nc) as tc:
        with (
            tc.tile_pool(name="wpool", bufs=1) as wpool,
            tc.tile_pool(name="xpool", bufs=6) as xpool,
            tc.tile_pool(name="ppool", bufs=4, space="PSUM") as ppool,
            tc.tile_pool(name="spool", bufs=3) as spool,
            tc.tile_pool(name="tpool", bufs=3) as tpool,
            tc.tile_pool(name="opool", bufs=1) as opool,
        ):
            # DMA issue order: w16 half 0, xh/u8 tile 0, v8, w16 half 1 —
            # so tile-0 main matmuls and the fp8 pass start early.
            KS = KCH // 2
            w16a = wpool.tile([128, KS * E], dt.float16, tag="w16a")
            w16b = wpool.tile([128, KS * E], dt.float16, tag="w16b")
            w16_flat = w16_d[:].rearrange("p k e -> p (k e)")
            for s in range(2):
                sz = KS * E // 2
                nc.sync.dma_start(w16a[:, s * sz:(s + 1) * sz],
                                  w16_flat[:, s * sz:(s + 1) * sz])
            x0 = xpool.tile([128, KCH * 128], dt.float16, tag="x")
            nc.sync.dma_start(x0[:], xh_d[0].rearrange("p k t -> p (k t)"))
            u0 = xpool.tile([128, SCH * 128], dt.float8e4, tag="u")
            nc.sync.dma_start(u0[:], u8_d[0].rearrange("p s t -> p (s t)"))
            v8 = wpool.tile([128, SCH * E], dt.float8e4, tag="v8")
            v8_flat = v8_d[:].rearrange("p s e -> p (s e)")
            for s in range(4):
                sz = SCH * E // 4
                nc.sync.dma_start(v8[:, s * sz:(s + 1) * sz],
                                  v8_flat[:, s * sz:(s + 1) * sz])
            for s in range(2):
                sz = KS * E // 2
                nc.sync.dma_start(w16b[:, s * sz:(s + 1) * sz],
                                  w16_flat[:, KS * E + s * sz:KS * E + (s + 1) * sz])
            btab = wpool.tile([128, E], dt.float32, tag="btab")
            nc.sync.dma_start(btab[:], btab_d[:])

            out_w = opool.tile([128, NTILES * K], dt.float32, tag="ow")
            out_i = opool.tile([128, NTILES * K], dt.uint32, tag="oi")

            for i in range(NTILES):
                if i == 0:
                    x, u = x0, u0
                else:
                    x = xpool.tile([128, KCH * 128], dt.float16, tag="x")
                    nc.sync.dma_start(x[:], xh_d[i].rearrange("p k t -> p (k t)"))
                    u = xpool.tile([128, SCH * 128], dt.float8e4, tag="u")
                    nc.sync.dma_start(u[:], u8_d[i].rearrange("p s t -> p (s t)"))

                psm = ppool.tile([128, E], dt.float32, tag="psm")
                for k in range(KCH):
                    wt = w16a if k < KS else w16b
                    kk = k % KS
                    nc.tensor.matmul(psm[:],
                                     lhsT=x[:, k * 128:(k + 1) * 128],
                                     rhs=wt[:, kk * E:(kk + 1) * E],
                                     start=(k == 0), stop=(k == KCH - 1))

                psc = ppool.tile([128, E], dt.float32, tag="psc")
                u3 = u[:].rearrange("p (s t) -> p s t", s=SCH)
                v3 = v8[:].rearrange("p (s e) -> p s e", s=SCH)
                for j in range(SCH // 2):
                    nc.tensor.matmul(psc[:],
                                     lhsT=u3[:, 2 * j:2 * j + 2, :],
                                     rhs=v3[:, 2 * j:2 * j + 2, :],
                                     start=(j == 0), stop=(j == SCH // 2 - 1),
                                     perf_mode=DR)

                # --- combine + routing epilogue (ACT + GPSIMD + DVE) ---
                # DVE may read only one PSUM operand: ACT rescales psc to SBUF first
                s2 = spool.tile([128, E], dt.float32, tag="s2")
                nc.scalar.mul(s2[:], psc[:], CSCALE)
                comb = spool.tile([128, E], dt.float32, tag="comb")
                nc.vector.tensor_tensor(comb[:], s2[:], psm[:], Alu.add)
                scores = spool.tile([128, E], dt.float32, tag="scores")
                nc.scalar.activation(scores[:], comb[:], Act.Sigmoid)

                sr = spool.tile([128, E], dt.float32, tag="sr")
                nc.gpsimd.tensor_tensor(sr[:], scores[:], btab[:], Alu.add)
                sr3 = sr[:].rearrange("p (g e) -> p g e", g=G)

                top1 = tpool.tile([128, G], dt.float32, tag="top1")
                nc.vector.tensor_reduce(top1[:], sr3, axis=Ax.X, op=Alu.max)
                mr2 = spool.tile([128, E], dt.float32, tag="mr2")
                nc.vector.match_replace(mr2[:], in_to_replace=top1[:], in_values=sr[:], imm_value=NEG)
                top2 = tpool.tile([128, G], dt.float32, tag="top2")
                nc.vector.tensor_reduce(top2[:], mr2[:].rearrange("p (g e) -> p g e", g=G), axis=Ax.X, op=Alu.max)
                gs_t = tpool.tile([128, G], dt.float32, tag="gs")
                nc.vector.tensor_tensor(gs_t[:], top1[:], top2[:], Alu.add)
                g8 = tpool.tile([128, 8], dt.float32, tag="g8")
                nc.vector.max(out=g8[:], in_=gs_t[:])
                inv = tpool.tile([128, G], dt.float32, tag="inv")
                nc.vector.tensor_scalar(inv[:], gs_t[:], g8[:, 3:4], -NEG, op0=Alu.is_lt, op1=Alu.mult)
                # mask: sr -= inv (0 for kept groups, 1e4 for dropped)
                nc.gpsimd.tensor_tensor(sr3, sr3, inv[:].to_broadcast([128, G, GS]), Alu.subtract)

                vals8 = tpool.tile([128, K], dt.float32, tag="vals8")
                nc.vector.max(out=vals8[:], in_=sr[:])
                idx8 = tpool.tile([128, K], dt.uint32, tag="idx8")
                nc.vector.max_index(out=idx8[:], in_max=vals8[:], in_values=sr[:])

                # selected positions -> exactly BIG; (BIG * 2^-100) * score = score
                mr = spool.tile([128, E], dt.float32, tag="mr")
                nc.vector.match_replace(mr[:], in_to_replace=vals8[:], in_values=sr[:], imm_value=BIG)
                sel_s = spool.tile([128, E], dt.float32, tag="sel_s")
                nc.vector.scalar_tensor_tensor(sel_s[:], in0=mr[:], scalar=2.0 ** -100,
                                               in1=scores[:], op0=Alu.mult, op1=Alu.mult)

                svals8 = tpool.tile([128, K], dt.float32, tag="svals8")
                nc.vector.max(out=svals8[:], in_=sel_s[:])
                sidx8 = tpool.tile([128, K], dt.uint32, tag="sidx8")
                nc.vector.max_index(out=sidx8[:], in_max=svals8[:], in_values=sel_s[:])

                idx8f = tpool.tile([128, K], dt.float32, tag="idx8f")
                nc.vector.tensor_copy(idx8f[:], idx8[:])
                sidx8f = tpool.tile([128, K], dt.float32, tag="sidx8f")
                nc.vector.tensor_copy(sidx8f[:], sidx8[:])

                # reorder svals8 (score order) into idx8 (routing order): K x K match
                eq = tpool.tile([128, K * K], dt.float32, tag="eq")
                eq3 = eq[:].rearrange("p (k j) -> p k j", k=K)
                nc.vector.tensor_tensor(eq3, idx8f[:].to_broadcast([128, K, K]), bc_mid(sidx8f[:]), Alu.is_equal)
                prod = tpool.tile([128, K * K], dt.float32, tag="prod")
                prod3 = prod[:].rearrange("p (k j) -> p k j", k=K)
                nc.vector.tensor_tensor(prod3, eq3, bc_mid(svals8[:]), Alu.mult)
                w8 = tpool.tile([128, K], dt.float32, tag="w8")
                nc.vector.tensor_reduce(w8[:], prod3, axis=Ax.X, op=Alu.add)

                sum8 = tpool.tile([128, 1], dt.float32, tag="sum8")
                nc.vector.tensor_reduce(sum8[:], w8[:], axis=Ax.X, op=Alu.add)
                rec = tpool.tile([128, 1], dt.float32, tag="rec")
                nc.vector.reciprocal(rec[:], sum8[:])

                nc.vector.tensor_scalar(out_w[:, i * K:(i + 1) * K], w8[:], rec[:, 0:1], 2.5,
                                        op0=Alu.mult, op1=Alu.mult)
                nc.gpsimd.tensor_copy(out_i[:, i * K:(i + 1) * K], idx8[:])

                # stream results out every 4 tiles so the final DMA is tiny
                if i % 4 == 3:
                    g = i // 4
                    wo4 = w_out[:].rearrange("(i p) k -> p i k", p=128)[:, 4 * g:4 * g + 4, :]
                    io4 = i_out[:].rearrange("(i p) k -> p i k", p=128)[:, 4 * g:4 * g + 4, :]
                    nc.sync.dma_start(wo4, out_w[:, 32 * g:32 * (g + 1)].rearrange("p (i k) -> p i k", i=4))
                    nc.sync.dma_start(io4, out_i[:, 32 * g:32 * (g + 1)].rearrange("p (i k) -> p i k", i=4))

    nc.compile()
    return nc


def _prep(hidden_states, weight, expert_bias):
    import ml_dtypes
    f8 = ml_dtypes.float8_e4m3
    x = np.ascontiguousarray(hidden_states, dtype=np.float32)
    w = np.ascontiguousarray(weight, dtype=np.float32)

    w16 = w.astype(np.float16)
    wl = w - w16.astype(np.float32)
    wl8 = (wl * 2.0 ** 19).astype(f8)       # scaled residual of w
    wh8 = (w * 2.0 ** 7).astype(f8)         # scaled w
    # [256, 4096] -> [128p, k, 256e] layouts
    w16_l = np.ascontiguousarray(w16.reshape(E, KCH, 128).transpose(2, 1, 0))
    v8 = np.concatenate([wl8.reshape(E, KCH, 128), wh8.reshape(E, KCH, 128)], axis=1)
    v8_l = np.ascontiguousarray(v8.transpose(2, 1, 0))   # [128, 64, 256]
    btab = np.ascontiguousarray(np.broadcast_to(expert_bias.astype(np.float32), (128, E)))

    in_maps = []
    for c in range(NCORES):
        xs = x[c * TPC:(c + 1) * TPC]
        xh16 = xs.astype(np.float16)
        xl = xs - xh16.astype(np.float32)
        xh8 = xs.astype(f8)
        xl8 = (xl * 2.0 ** 12).astype(f8)
        # [2048, 4096] -> [16i, 128p(h), k, 128t]
        xh_l = np.ascontiguousarray(xh16.reshape(NTILES, 128, KCH, 128).transpose(0, 3, 2, 1))
        u8 = np.concatenate([xh8.reshape(NTILES, 128, KCH, 128),
                             xl8.reshape(NTILES, 128, KCH, 128)], axis=2)
        u8_l = np.ascontiguousarray(u8.transpose(0, 3, 2, 1))  # [16, 128, 64, 128]
        in_maps.append({"xh": xh_l, "u8": u8_l, "w16": w16_l, "v8": v8_l, "btab": btab})
    return in_maps


def kernel(hidden_states, weight, expert_bias, _trace=False):
    from concourse.bass_utils import run_bass_kernel_spmd

    if "nc" not in _cache:
        _cache["nc"] = _build()
    nc = _cache["nc"]
    in_maps = _prep(hidden_states, weight, expert_bias)
    res = run_bass_kernel_spmd(nc, in_maps, core_ids=list(range(NCORES)), trace=_trace)
    _cache["last_results"] = res
    w = np.concatenate([res.results[c]["w_out"] for c in range(NCORES)], axis=0)
    idx = np.concatenate([res.results[c]["i_out"] for c in range(NCORES)], axis=0)
    return w.astype(np.float32), idx.astype(np.int32)


# revision 16
# speedup vs baseline: 1.1718x; 1.0489x over previous
"""MoE gate (LLaDA2) routing kernel for 8 Trainium2 NeuronCores.

Strategy: token-parallel over 8 cores (2048 tokens/core). Router GEMM split
into a fp16 main pass (xh16 @ w16) plus a single fp8e4m3 DoubleRow GEMM that
computes BOTH cross-correction terms (xh@wl + xl@wh) via augmented vectors
u=[x|xl*2^12], v=[wl*2^19|w*2^7] over a 8192-long contraction at 0.5
cycles/row — recovering ~15.5-bit effective operand precision (needed for
exact top-k ordering) at ~60% of the PE time of an fp16 3-term split.
Grouped top-k routing on-chip: DVE max8/max_index/match_replace with cheap
elementwise stages offloaded to GPSIMD.
"""
import sys
for p in ("/opt/trn_rl_repo", "/root/.axon_site/_ro/trn_rl_repo"):
    if p not in sys.path:
        sys.path.append(p)

import numpy as np

T, H, E = 16384, 4096, 256
NCORES = 8
TPC = T // NCORES          # tokens per core: 2048
NTILES = TPC // 128        # 16 row tiles
KCH = H // 128             # 32 contraction chunks (fp16 main pass)
SCH = 2 * KCH              # 64 fp8 subchunks (augmented contraction)
G = 8                      # expert groups
GS = E // G                # 32 experts/group
K = 8                      # top-k
BIG = 2.0 ** 100
NEG = -1.0e4
CSCALE = 2.0 ** -19        # undo fp8 segment scaling at combine time

_cache = {}


def _build():
    import concourse.bacc as bacc
    import concourse.bass as bass
    import concourse.mybir as mybir
    from concourse import tile

    dt = mybir.dt
    Alu = mybir.AluOpType
    Act = mybir.ActivationFunctionType
    Ax = mybir.AxisListType
    DR = mybir.MatmulPerfMode.DoubleRow

    nc = bacc.Bacc("TRN2", target_bir_lowering=False, debug=False,
                   num_devices=NCORES)

    xh_d = nc.dram_tensor("xh", [NTILES, 128, KCH, 128], dt.float16, kind="ExternalInput")
    u8_d = nc.dram_tensor("u8", [NTILES, 128, SCH, 128], dt.float8e4, kind="ExternalInput")
    w16_d = nc.dram_tensor("w16", [128, KCH, E], dt.float16, kind="ExternalInput")
    v8_d = nc.dram_tensor("v8", [128, SCH, E], dt.float8e4, kind="ExternalInput")
    btab_d = nc.dram_tensor("btab", [128, E], dt.float32, kind="ExternalInput")
    w_out = nc.dram_tensor("w_out", [TPC, K], dt.float32, kind="ExternalOutput")
    i_out = nc.dram_tensor("i_out", [TPC, K], dt.uint32, kind="ExternalOutput")

    def bc_mid(ap8, n=8):
        # [128, m] -> [128, n(bcast), m]
        return bass.AP(ap8.tensor, ap8.offset, [list(ap8.ap[0]), [0, n], list(ap8.ap[1])])

    with tile.TileContext(nc) as tc:
        with (
            tc.tile_pool(name="wpool", bufs=1) as wpool,
            tc.tile_pool(name="xpool", bufs=6) as xpool,
            tc.tile_pool(name="ppool", bufs=4, space="PSUM") as ppool,
            tc.tile_pool(name="spool", bufs=3) as spool,
            tc.tile_pool(name="tpool", bufs=3) as tpool,
            tc.tile_pool(name="opool", bufs=1) as opool,
        ):
            # DMA issue order: w16 half 0, xh/u8 tile 0, v8, w16 half 1 —
            # so tile-0 main matmuls and the fp8 pass start early.
            KS = KCH // 2
            w16a = wpool.tile([128, KS * E], dt.float16, tag="w16a")
            w16b = wpool.tile([128, KS * E], dt.float16, tag="w16b")
            w16_flat = w16_d[:].rearrange("p k e -> p (k e)")
            for s in range(2):
                sz = KS * E // 2
                nc.sync.dma_start(w16a[:, s * sz:(s + 1) * sz],
                                  w16_flat[:, s * sz:(s + 1) * sz])
            x0 = xpool.tile([128, KCH * 128], dt.float16, tag="x")
            nc.sync.dma_start(x0[:], xh_d[0].rearrange("p k t -> p (k t)"))
            u0 = xpool.tile([128, SCH * 128], dt.float8e4, tag="u")
            nc.sync.dma_start(u0[:], u8_d[0].rearrange("p s t -> p (s t)"))
            v8 = wpool.tile([128, SCH * E], dt.float8e4, tag="v8")
            v8_flat = v8_d[:].rearrange("p s e -> p (s e)")
            for s in range(4):
                sz = SCH * E // 4
                nc.sync.dma_start(v8[:, s * sz:(s + 1) * sz],
                                  v8_flat[:, s * sz:(s + 1) * sz])
            for s in range(2):
                sz = KS * E // 2
                nc.sync.dma_start(w16b[:, s * sz:(s + 1) * sz],
                                  w16_flat[:, KS * E + s * sz:KS * E + (s + 1) * sz])
            btab = wpool.tile([128, E], dt.float32, tag="btab")
            nc.sync.dma_start(btab[:], btab_d[:])

            out_w = opool.tile([128, NTILES * K], dt.float32, tag="ow")
            out_i = opool.tile([128, NTILES * K], dt.uint32, tag="oi")

            for i in range(NTILES):
                if i == 0:
                    x, u = x0, u0
                else:
                    x = xpool.tile([128, KCH * 128], dt.float16, tag="x")
                    nc.sync.dma_start(x[:], xh_d[i].rearrange("p k t -> p (k t)"))
                    u = xpool.tile([128, SCH * 128], dt.float8e4, tag="u")
                    nc.sync.dma_start(u[:], u8_d[i].rearrange("p s t -> p (s t)"))

                psm = ppool.tile([128, E], dt.float32, tag="psm")
                for k in range(KCH):
                    wt = w16a if k < KS else w16b
                    kk = k % KS
                    nc.tensor.matmul(psm[:],
                                     lhsT=x[:, k * 128:(k + 1) * 128],
                                     rhs=wt[:, kk * E:(kk + 1) * E],
                                     start=(k == 0), stop=(k == KCH - 1))

                psc = ppool.tile([128, E], dt.float32, tag="psc")
                u3 = u[:].rearrange("p (s t) -> p s t", s=SCH)
                v3 = v8[:].rearrange("p (s e) -> p s e", s=SCH)
                for j in range(SCH // 2):
                    nc.tensor.matmul(psc[:],
                                     lhsT=u3[:, 2 * j:2 * j + 2, :],
                                     rhs=v3[:, 2 * j:2 * j + 2, :],
                                     start=(j == 0), stop=(j == SCH // 2 - 1),
                                     perf_mode=DR)

                # --- combine + routing epilogue (ACT + GPSIMD + DVE) ---
                # DVE may read only one PSUM operand: ACT rescales psc to SBUF first
                s2 = spool.tile([128, E], dt.float32, tag="s2")
                nc.scalar.mul(s2[:], psc[:], CSCALE)
                comb = spool.tile([128, E], dt.float32, tag="comb")
                nc.vector.tensor_tensor(comb[:], s2[:], psm[:], Alu.add)
                scores = spool.tile([128, E], dt.float32, tag="scores")
                nc.scalar.activation(scores[:], comb[:], Act.Sigmoid)

                sr = spool.tile([128, E], dt.float32, tag="sr")
                nc.gpsimd.tensor_tensor(sr[:], scores[:], btab[:], Alu.add)
                sr3 = sr[:].rearrange("p (g e) -> p g e", g=G)

                top1 = tpool.tile([128, G], dt.float32, tag="top1")
                nc.vector.tensor_reduce(top1[:], sr3, axis=Ax.X, op=Alu.max)
                mr2 = spool.tile([128, E], dt.float32, tag="mr2")
                nc.vector.match_replace(mr2[:], in_to_replace=top1[:], in_values=sr[:], imm_value=NEG)
                top2 = tpool.tile([128, G], dt.float32, tag="top2")
                nc.vector.tensor_reduce(top2[:], mr2[:].rearrange("p (g e) -> p g e", g=G), axis=Ax.X, op=Alu.max)
                gs_t = tpool.tile([128, G], dt.float32, tag="gs")
                nc.vector.tensor_tensor(gs_t[:], top1[:], top2[:], Alu.add)
                g8 = tpool.tile([128, 8], dt.float32, tag="g8")
                nc.vector.max(out=g8[:], in_=gs_t[:])
                inv = tpool.tile([128, G], dt.float32, tag="inv")
                nc.vector.tensor_scalar(inv[:], gs_t[:], g8[:, 3:4], -NEG, op0=Alu.is_lt, op1=Alu.mult)
                # mask: sr -= inv (0 for kept groups, 1e4 for dropped)
                nc.gpsimd.tensor_tensor(sr3, sr3, inv[:].to_broadcast([128, G, GS]), Alu.subtract)

                vals8 = tpool.tile([128, K], dt.float32, tag="vals8")
                nc.vector.max(out=vals8[:], in_=sr[:])
                idx8 = tpool.tile([128, K], dt.uint32, tag="idx8")
                nc.vector.max_index(out=idx8[:], in_max=vals8[:], in_values=sr[:])

                # selected positions -> exactly BIG; (BIG * 2^-100) * score = score
                mr = spool.tile([128, E], dt.float32, tag="mr")
                nc.vector.match_replace(mr[:], in_to_replace=vals8[:], in_values=sr[:], imm_value=BIG)
                sel_s = spool.tile([128, E], dt.float32, tag="sel_s")
                nc.vector.scalar_tensor_tensor(sel_s[:], in0=mr[:], scalar=2.0 ** -100,
                                               in1=scores[:], op0=Alu.mult, op1=Alu.mult)

                svals8 = tpool.tile([128, K], dt.float32, tag="svals8")
                nc.vector.max(out=svals8[:], in_=sel_s[:])
                sidx8 = tpool.tile([128, K], dt.uint32, tag="sidx8")
                nc.vector.max_index(out=sidx8[:], in_max=svals8[:], in_values=sel_s[:])

                idx8f = tpool.tile([128, K], dt.float32, tag="idx8f")
                nc.vector.tensor_copy(idx8f[:], idx8[:])
                sidx8f = tpool.tile([128, K], dt.float32, tag="sidx8f")
                nc.vector.tensor_copy(sidx8f[:], sidx8[:])

                # reorder svals8 (score order) into idx8 (routing order): K x K match
                eq = tpool.tile([128, K * K], dt.float32, tag="eq")
                eq3 = eq[:].rearrange("p (k j) -> p k j", k=K)
                nc.vector.tensor_tensor(eq3, idx8f[:].to_broadcast([128, K, K]), bc_mid(sidx8f[:]), Alu.is_equal)
                prod = tpool.tile([128, K * K], dt.float32, tag="prod")
                prod3 = prod[:].rearrange("p (k j) -> p k j", k=K)
                nc.vector.tensor_tensor(prod3, eq3, bc_mid(svals8[:]), Alu.mult)
                w8 = tpool.tile([128, K], dt.float32, tag="w8")
                nc.vector.tensor_reduce(w8[:], prod3, axis=Ax.X, op=Alu.add)

                sum8 = tpool.tile([128, 1], dt.float32, tag="sum8")
                nc.vector.tensor_reduce(sum8[:], w8[:], axis=Ax.X, op=Alu.add)
                rec = tpool.tile([128, 1], dt.float32, tag="rec")
                nc.vector.reciprocal(rec[:], sum8[:])

                nc.vector.tensor_scalar(out_w[:, i * K:(i + 1) * K], w8[:], rec[:, 0:1], 2.5,
                                        op0=Alu.mult, op1=Alu.mult)
                nc.gpsimd.tensor_copy(out_i[:, i * K:(i + 1) * K], idx8[:])

            # issued after all input DMAs so they cannot head-of-line block them;
            # 4 gate groups let finished tiles stream out before the last epilogue
            for g in range(4):
                wo4 = w_out[:].rearrange("(i p) k -> p i k", p=128)[:, 4 * g:4 * g + 4, :]
                io4 = i_out[:].rearrange("(i p) k -> p i k", p=128)[:, 4 * g:4 * g + 4, :]
                nc.sync.dma_start(wo4, out_w[:, 32 * g:32 * (g + 1)].rearrange("p (i k) -> p i k", i=4))
                nc.sync.dma_start(io4, out_i[:, 32 * g:32 * (g + 1)].rearrange("p (i k) -> p i k", i=4))

    nc.compile()
    return nc


def _prep(hidden_states, weight, expert_bias):
    import ml_dtypes
    f8 = ml_dtypes.float8_e4m3
    x = np.ascontiguousarray(hidden_states, dtype=np.float32)
    w = np.ascontiguousarray(weight, dtype=np.float32)

    w16 = w.astype(np.float16)
    wl = w - w16.astype(np.float32)
    wl8 = (wl * 2.0 ** 19).astype(f8)       # scaled residual of w
    wh8 = (w * 2.0 ** 7).astype(f8)         # scaled w
    # [256, 4096] -> [128p, k, 256e] layouts
    w16_l = np.ascontiguousarray(w16.reshape(E, KCH, 128).transpose(2, 1, 0))
    v8 = np.concatenate([wl8.reshape(E, KCH, 128), wh8.reshape(E, KCH, 128)], axis=1)
    v8_l = np.ascontiguousarray(v8.transpose(2, 1, 0))   # [128, 64, 256]
    btab = np.ascontiguousarray(np.broadcast_to(expert_bias.astype(np.float32), (128, E)))

    in_maps = []
    for c in range(NCORES):
        xs = x[c * TPC:(c + 1) * TPC]
        xh16 = xs.astype(np.float16)
        xl = xs - xh16.astype(np.float32)
        xh8 = xs.astype(f8)
        xl8 = (xl * 2.0 ** 12).astype(f8)
        # [2048, 4096] -> [16i, 128p(h), k, 128t]
        xh_l = np.ascontiguousarray(xh16.reshape(NTILES, 128, KCH, 128).transpose(0, 3, 2, 1))
        u8 = np.concatenate([xh8.reshape(NTILES, 128, KCH, 128),
                             xl8.reshape(NTILES, 128, KCH, 128)], axis=2)
        u8_l = np.ascontiguousarray(u8.transpose(0, 3, 2, 1))  # [16, 128, 64, 128]
        in_maps.append({"xh": xh_l, "u8": u8_l, "w16": w16_l, "v8": v8_l, "btab": btab})
    return in_maps


def kernel(hidden_states, weight, expert_bias, _trace=False):
    from concourse.bass_utils import run_bass_kernel_spmd

    if "nc" not in _cache:
        _cache["nc"] = _build()
    nc = _cache["nc"]
    in_maps = _prep(hidden_states, weight, expert_bias)
    res = run_bass_kernel_spmd(nc, in_maps, core_ids=list(range(NCORES)), trace=_trace)
    _cache["last_results"] = res
    w = np.concatenate([res.results[c]["w_out"] for c in range(NCORES)], axis=0)
    idx = np.concatenate([res.results[c]["i_out"] for c in range(NCORES)], axis=0)
    return w.astype(np.float32), idx.astype(np.int32)


# revision 17
# speedup vs baseline: 1.1843x; 1.0107x over previous
"""MoE gate (LLaDA2) routing kernel for 8 Trainium2 NeuronCores.

Strategy: token-parallel over 8 cores (2048 tokens/core). Router GEMM split
into a fp16 main pass (xh16 @ w16) plus a single fp8e4m3 DoubleRow GEMM that
computes BOTH cross-correction terms (xh@wl + xl@wh) via augmented vectors
u=[x|xl*2^12], v=[wl*2^19|w*2^7] over a 8192-long contraction at 0.5
cycles/row — recovering ~15.5-bit effective operand precision (needed for
exact top-k ordering) at ~60% of the PE time of an fp16 3-term split.
Grouped top-k routing on-chip: DVE max8/max_index/match_replace with cheap
elementwise stages offloaded to GPSIMD.
"""
import sys
for p in ("/opt/trn_rl_repo", "/root/.axon_site/_ro/trn_rl_repo"):
    if p not in sys.path:
        sys.path.append(p)

import numpy as np

T, H, E = 16384, 4096, 256
NCORES = 8
TPC = T // NCORES          # tokens per core: 2048
NTILES = TPC // 128        # 16 row tiles
KCH = H // 128             # 32 contraction chunks (fp16 main pass)
SCH = 2 * KCH              # 64 fp8 subchunks (augmented contraction)
G = 8                      # expert groups
GS = E // G                # 32 experts/group
K = 8                      # top-k
BIG = 2.0 ** 100
NEG = -1.0e4
CSCALE = 2.0 ** -19        # undo fp8 segment scaling at combine time

_cache = {}


def _build():
    import concourse.bacc as bacc
    import concourse.bass as bass
    import concourse.mybir as mybir
    from concourse import tile

    dt = mybir.dt
    Alu = mybir.AluOpType
    Act = mybir.ActivationFunctionType
    Ax = mybir.AxisListType
    DR = mybir.MatmulPerfMode.DoubleRow

    nc = bacc.Bacc("TRN2", target_bir_lowering=False, debug=False,
                   num_devices=NCORES)

    xh_d = nc.dram_tensor("xh", [NTILES, 128, KCH, 128], dt.float16, kind="ExternalInput")
    u8_d = nc.dram_tensor("u8", [NTILES, 128, SCH, 128], dt.float8e4, kind="ExternalInput")
    w16_d = nc.dram_tensor("w16", [128, KCH, E], dt.float16, kind="ExternalInput")
    v8_d = nc.dram_tensor("v8", [128, SCH, E], dt.float8e4, kind="ExternalInput")
    btab_d = nc.dram_tensor("btab", [128, E], dt.float32, kind="ExternalInput")
    w_out = nc.dram_tensor("w_out", [TPC, K], dt.float32, kind="ExternalOutput")
    i_out = nc.dram_tensor("i_out", [TPC, K], dt.uint32, kind="ExternalOutput")

    def bc_mid(ap8, n=8):
        # [128, m] -> [128, n(bcast), m]
        return bass.AP(ap8.tensor, ap8.offset, [list(ap8.ap[0]), [0, n], list(ap8.ap[1])])

    with tile.TileContext(nc) as tc:
        with (
            tc.tile_pool(name="wpool", bufs=1) as wpool,
            tc.tile_pool(name="xpool", bufs=6) as xpool,
            tc.tile_pool(name="ppool", bufs=4, space="PSUM") as ppool,
            tc.tile_pool(name="spool", bufs=4) as spool,
            tc.tile_pool(name="tpool", bufs=4) as tpool,
            tc.tile_pool(name="opool", bufs=1) as opool,
        ):
            # DMA issue order: w16 half 0, xh/u8 tile 0, v8, w16 half 1 —
            # so tile-0 main matmuls and the fp8 pass start early.
            KS = KCH // 2
            w16a = wpool.tile([128, KS * E], dt.float16, tag="w16a")
            w16b = wpool.tile([128, KS * E], dt.float16, tag="w16b")
            w16_flat = w16_d[:].rearrange("p k e -> p (k e)")
            for s in range(2):
                sz = KS * E // 2
                nc.sync.dma_start(w16a[:, s * sz:(s + 1) * sz],
                                  w16_flat[:, s * sz:(s + 1) * sz])
            x0 = xpool.tile([128, KCH * 128], dt.float16, tag="x")
            nc.sync.dma_start(x0[:], xh_d[0].rearrange("p k t -> p (k t)"))
            u0 = xpool.tile([128, SCH * 128], dt.float8e4, tag="u")
            nc.sync.dma_start(u0[:], u8_d[0].rearrange("p s t -> p (s t)"))
            v8 = wpool.tile([128, SCH * E], dt.float8e4, tag="v8")
            v8_flat = v8_d[:].rearrange("p s e -> p (s e)")
            for s in range(4):
                sz = SCH * E // 4
                nc.sync.dma_start(v8[:, s * sz:(s + 1) * sz],
                                  v8_flat[:, s * sz:(s + 1) * sz])
            for s in range(2):
                sz = KS * E // 2
                nc.sync.dma_start(w16b[:, s * sz:(s + 1) * sz],
                                  w16_flat[:, KS * E + s * sz:KS * E + (s + 1) * sz])
            btab = wpool.tile([128, E], dt.float32, tag="btab")
            nc.sync.dma_start(btab[:], btab_d[:])

            out_w = opool.tile([128, NTILES * K], dt.float32, tag="ow")
            out_i = opool.tile([128, NTILES * K], dt.uint32, tag="oi")

            for i in range(NTILES):
                if i == 0:
                    x, u = x0, u0
                else:
                    x = xpool.tile([128, KCH * 128], dt.float16, tag="x")
                    nc.sync.dma_start(x[:], xh_d[i].rearrange("p k t -> p (k t)"))
                    u = xpool.tile([128, SCH * 128], dt.float8e4, tag="u")
                    nc.sync.dma_start(u[:], u8_d[i].rearrange("p s t -> p (s t)"))

                psm = ppool.tile([128, E], dt.float32, tag="psm")
                for k in range(KCH):
                    wt = w16a if k < KS else w16b
                    kk = k % KS
                    nc.tensor.matmul(psm[:],
                                     lhsT=x[:, k * 128:(k + 1) * 128],
                                     rhs=wt[:, kk * E:(kk + 1) * E],
                                     start=(k == 0), stop=(k == KCH - 1))

                psc = ppool.tile([128, E], dt.float32, tag="psc")
                u3 = u[:].rearrange("p (s t) -> p s t", s=SCH)
                v3 = v8[:].rearrange("p (s e) -> p s e", s=SCH)
                for j in range(SCH // 2):
                    nc.tensor.matmul(psc[:],
                                     lhsT=u3[:, 2 * j:2 * j + 2, :],
                                     rhs=v3[:, 2 * j:2 * j + 2, :],
                                     start=(j == 0), stop=(j == SCH // 2 - 1),
                                     perf_mode=DR)

                # --- combine + routing epilogue (ACT + GPSIMD + DVE) ---
                # DVE may read only one PSUM operand: ACT rescales psc to SBUF first
                s2 = spool.tile([128, E], dt.float32, tag="s2")
                nc.scalar.mul(s2[:], psc[:], CSCALE)
                comb = spool.tile([128, E], dt.float32, tag="comb")
                nc.vector.tensor_tensor(comb[:], s2[:], psm[:], Alu.add)
                scores = spool.tile([128, E], dt.float32, tag="scores")
                nc.scalar.activation(scores[:], comb[:], Act.Sigmoid)

                sr = spool.tile([128, E], dt.float32, tag="sr")
                nc.gpsimd.tensor_tensor(sr[:], scores[:], btab[:], Alu.add)
                sr3 = sr[:].rearrange("p (g e) -> p g e", g=G)

                top1 = tpool.tile([128, G], dt.float32, tag="top1")
                nc.vector.tensor_reduce(top1[:], sr3, axis=Ax.X, op=Alu.max)
                mr2 = spool.tile([128, E], dt.float32, tag="mr2")
                nc.vector.match_replace(mr2[:], in_to_replace=top1[:], in_values=sr[:], imm_value=NEG)
                top2 = tpool.tile([128, G], dt.float32, tag="top2")
                nc.vector.tensor_reduce(top2[:], mr2[:].rearrange("p (g e) -> p g e", g=G), axis=Ax.X, op=Alu.max)
                gs_t = tpool.tile([128, G], dt.float32, tag="gs")
                nc.vector.tensor_tensor(gs_t[:], top1[:], top2[:], Alu.add)
                g8 = tpool.tile([128, 8], dt.float32, tag="g8")
                nc.vector.max(out=g8[:], in_=gs_t[:])
                inv = tpool.tile([128, G], dt.float32, tag="inv")
                nc.vector.tensor_scalar(inv[:], gs_t[:], g8[:, 3:4], -NEG, op0=Alu.is_lt, op1=Alu.mult)
                # mask: sr -= inv (0 for kept groups, 1e4 for dropped)
                nc.gpsimd.tensor_tensor(sr3, sr3, inv[:].to_broadcast([128, G, GS]), Alu.subtract)

                vals8 = tpool.tile([128, K], dt.float32, tag="vals8")
                nc.vector.max(out=vals8[:], in_=sr[:])
                idx8 = tpool.tile([128, K], dt.uint32, tag="idx8")
                nc.vector.max_index(out=idx8[:], in_max=vals8[:], in_values=sr[:])

                # selected positions -> exactly BIG; (BIG * 2^-100) * score = score
                mr = spool.tile([128, E], dt.float32, tag="mr")
                nc.vector.match_replace(mr[:], in_to_replace=vals8[:], in_values=sr[:], imm_value=BIG)
                sel_s = spool.tile([128, E], dt.float32, tag="sel_s")
                nc.vector.scalar_tensor_tensor(sel_s[:], in0=mr[:], scalar=2.0 ** -100,
                                               in1=scores[:], op0=Alu.mult, op1=Alu.mult)

                svals8 = tpool.tile([128, K], dt.float32, tag="svals8")
                nc.vector.max(out=svals8[:], in_=sel_s[:])
                sidx8 = tpool.tile([128, K], dt.uint32, tag="sidx8")
                nc.vector.max_index(out=sidx8[:], in_max=svals8[:], in_values=sel_s[:])

                idx8f = tpool.tile([128, K], dt.float32, tag="idx8f")
                nc.vector.tensor_copy(idx8f[:], idx8[:])
                sidx8f = tpool.tile([128, K], dt.float32, tag="sidx8f")
                nc.vector.tensor_copy(sidx8f[:], sidx8[:])

                # reorder svals8 (score order) into idx8 (routing order): K x K match
                eq = tpool.tile([128, K * K], dt.float32, tag="eq")
                eq3 = eq[:].rearrange("p (k j) -> p k j", k=K)
                nc.vector.tensor_tensor(eq3, idx8f[:].to_broadcast([128, K, K]), bc_mid(sidx8f[:]), Alu.is_equal)
                prod = tpool.tile([128, K * K], dt.float32, tag="prod")
                prod3 = prod[:].rearrange("p (k j) -> p k j", k=K)
                nc.vector.tensor_tensor(prod3, eq3, bc_mid(svals8[:]), Alu.mult)
                w8 = tpool.tile([128, K], dt.float32, tag="w8")
                nc.vector.tensor_reduce(w8[:], prod3, axis=Ax.X, op=Alu.add)

                sum8 = tpool.tile([128, 1], dt.float32, tag="sum8")
                nc.vector.tensor_reduce(sum8[:], w8[:], axis=Ax.X, op=Alu.add)
                rec = tpool.tile([128, 1], dt.float32, tag="rec")
                nc.vector.reciprocal(rec[:], sum8[:])

                nc.vector.tensor_scalar(out_w[:, i * K:(i + 1) * K], w8[:], rec[:, 0:1], 2.5,
                                        op0=Alu.mult, op1=Alu.mult)
                nc.gpsimd.tensor_copy(out_i[:, i * K:(i + 1) * K], idx8[:])

            # issued after all input DMAs so they cannot head-of-line block them;
            # 4 gate groups let finished tiles stream out before the last epilogue
            for g in range(4):
                wo4 = w_out[:].rearrange("(i p) k -> p i k", p=128)[:, 4 * g:4 * g + 4, :]
                io4 = i_out[:].rearrange("(i p) k -> p i k", p=128)[:, 4 * g:4 * g + 4, :]
                nc.sync.dma_start(wo4, out_w[:, 32 * g:32 * (g + 1)].rearrange("p (i k) -> p i k", i=4))
                nc.sync.dma_start(io4, out_i[:, 32 * g:32 * (g + 1)].rearrange("p (i k) -> p i k", i=4))

    nc.compile()
    return nc


def _prep(hidden_states, weight, expert_bias):
    import ml_dtypes
    f8 = ml_dtypes.float8_e4m3
    x = np.ascontiguousarray(hidden_states, dtype=np.float32)
    w = np.ascontiguousarray(weight, dtype=np.float32)

    w16 = w.astype(np.float16)
    wl = w - w16.astype(np.float32)
    wl8 = (wl * 2.0 ** 19).astype(f8)       # scaled residual of w
    wh8 = (w * 2.0 ** 7).astype(f8)         # scaled w
    # [256, 4096] -> [128p, k, 256e] layouts
    w16_l = np.ascontiguousarray(w16.reshape(E, KCH, 128).transpose(2, 1, 0))
    v8 = np.concatenate([wl8.reshape(E, KCH, 128), wh8.reshape(E, KCH, 128)], axis=1)
    v8_l = np.ascontiguousarray(v8.transpose(2, 1, 0))   # [128, 64, 256]
    btab = np.ascontiguousarray(np.broadcast_to(expert_bias.astype(np.float32), (128, E)))

    in_maps = []
    for c in range(NCORES):
        xs = x[c * TPC:(c + 1) * TPC]
        xh16 = xs.astype(np.float16)
        xl = xs - xh16.astype(np.float32)
        xh8 = xs.astype(f8)
        xl8 = (xl * 2.0 ** 12).astype(f8)
        # [2048, 4096] -> [16i, 128p(h), k, 128t]
        xh_l = np.ascontiguousarray(xh16.reshape(NTILES, 128, KCH, 128).transpose(0, 3, 2, 1))
        u8 = np.concatenate([xh8.reshape(NTILES, 128, KCH, 128),
                             xl8.reshape(NTILES, 128, KCH, 128)], axis=2)
        u8_l = np.ascontiguousarray(u8.transpose(0, 3, 2, 1))  # [16, 128, 64, 128]
        in_maps.append({"xh": xh_l, "u8": u8_l, "w16": w16_l, "v8": v8_l, "btab": btab})
    return in_maps


def kernel(hidden_states, weight, expert_bias, _trace=False):
    from concourse.bass_utils import run_bass_kernel_spmd

    if "nc" not in _cache:
        _cache["nc"] = _build()
    nc = _cache["nc"]
    in_maps = _prep(hidden_states, weight, expert_bias)
    res = run_bass_kernel_spmd(nc, in_maps, core_ids=list(range(NCORES)), trace=_trace)
    _cache["last_results"] = res
    w = np.concatenate([res.results[c]["w_out"] for c in range(NCORES)], axis=0)
    idx = np.concatenate([res.results[c]["i_out"] for c in range(NCORES)], axis=0)
    return w.astype(np.float32), idx.astype(np.int32)
